# revision 21
# baseline (speedup 1.0000x reference)
"""Trainium2 Bass kernel for a 4-layer hierarchical-attention encoder.

Sharding: 8 cores = 2 batch groups x 4 sequence chunks of 512 query tokens.
Each core computes SA K/V projections for ITS OWN 512 tokens only; K and V'
are all-gathered (fp8) per batch group at each layer boundary, removing the
4x redundant K/V projection of the hidden-state-AllGather scheme.

Weights are fp8 (x32 host-side scale, folded into the existing bias/drain
ops); K/V'/gelu activations are fp8; the residual stream, Q and softmax
stay fp16/fp32. Softmax exp is batched 2 heads per ACT ([128,1024] PSUM
reads) with double-buffered score banks so the scalar engine pipelines
against the PE.
"""
import os
import sys

for _p in ("/root/.axon_site/_ro/trn_rl_repo", "/opt/trn_rl_repo", "/opt/pypackages",
           "/root/.axon_site/_ro/pypackages"):
    if os.path.isdir(_p) and _p not in sys.path:
        sys.path.append(_p)

import numpy as np

import concourse.bass as bass
import concourse.mybir as mybir
import concourse.tile as tile
from concourse import bacc
from concourse.bass_utils import run_bass_kernel_spmd

L, E, H, D, F = 4, 512, 8, 64, 2048
B, S, SK = 2, 2048, 1024
NCORES = 8
GROUPS = [[0, 1, 2, 3], [4, 5, 6, 7]]
CH = 512          # tokens per core
ET = E // 128     # 4 feature tiles
TT = CH // 128    # 4 token tiles in own chunk
FT = F // 128     # 16 ffn tiles
KT_SA = S // 128  # 16 key tiles (self)
KT_CA = SK // 128  # 8 key tiles (cross)
HW = 65           # head width incl. denominator column
WS = 1.0          # weights are fp16; no scale fold needed
IWS = 1.0 / WS
AGW = E + HW * H  # 1032 = K block cols + V' cols in the AG payload

FP32 = mybir.dt.float32
FP16 = mybir.dt.float16
FP8 = mybir.dt.float8e4
AF = mybir.ActivationFunctionType
OP = mybir.AluOpType

_CACHE = {}


def _build():
    nc = bacc.Bacc("TRN2", target_bir_lowering=False, debug=False, num_devices=NCORES)

    def din(name, shape, dt=FP16):
        return nc.dram_tensor(name, shape, dt, kind="ExternalInput").ap()

    own_fm0 = din("own_fm0", [E, CH])         # own chunk, feature-major
    own_tm0 = din("own_tm0", [CH, E])         # own chunk, token-major
    know_fm_d = din("know_fm", [E, SK], FP8)
    ident_d = din("ident", [128, 128])
    ones_d = din("ones", [1, 128])

    wqk_d = din("wqk", [L, ET, 128, 4 * E], FP16)       # q_sa|k_sa|q_ca|k_ca
    wvo_d = din("wvo", [L, ET, 128, 2 * HW * H + 2 * E], FP16)  # v_sa|v_ca|o_sa|o_ca
    w1_d = din("w1", [L, ET, 128, F], FP16)
    w2_d = din("w2", [L, FT, 128, E], FP16)
    rows_d = din("rows", [L, 1, 2 * HW * H + 3 * E])   # rbv_sa|rbv_ca|rbo_sa|rbo_ca|rb2
    bc_d = din("bc", [L, 128, 4 * ET + FT], FP32)      # bq_sa|bk_sa|bq_ca|bk_ca|b1
    lnr_d = din("lnr", [L, 1, 2 * E], FP16)            # g|b

    out_d = nc.dram_tensor("out_tm", [CH, E], FP16, kind="ExternalOutput").ap()

    # wqk segment offsets
    QS, KS, QC, KC = 0, E, 2 * E, 3 * E
    # wvo segment offsets
    VS, VC, OS, OC = 0, HW * H, 2 * HW * H, 2 * HW * H + E
    # rows segment offsets
    RVS, RVC, ROS, ROC, R2 = 0, HW * H, 2 * HW * H, 2 * HW * H + E, 2 * HW * H + 2 * E
    # bc columns
    BQS, BKS, BQC, BKC, B1 = 0, ET, 2 * ET, 3 * ET, 4 * ET

    with tile.TileContext(nc) as tc:
        from contextlib import ExitStack
        with ExitStack() as ctx:
            ep = ctx.enter_context
            const_p = ep(tc.tile_pool(name="const", bufs=1))
            know_p = ep(tc.tile_pool(name="know", bufs=4))
            kfm_p = ep(tc.tile_pool(name="kfm", bufs=4))      # [128,2048] SA K fp8
            kca_p = ep(tc.tile_pool(name="kca", bufs=6))      # [128,1024] CA K fp8
            vp_p = ep(tc.tile_pool(name="vp", bufs=24))       # V' fp8 [128,520]
            qfm_p = ep(tc.tile_pool(name="qfm", bufs=8))
            attn_p = ep(tc.tile_pool(name="attn", bufs=4))
            ofm_p = ep(tc.tile_pool(name="ofm", bufs=6))      # own_fm
            xfm_p = ep(tc.tile_pool(name="xfm", bufs=6))      # inter_fm / co_fm
            stm_p = ep(tc.tile_pool(name="stm", bufs=11))     # hid/inter/co TM fp16
            pt_p = ep(tc.tile_pool(name="pt", bufs=4))        # exp scores fp16 [128,1024]
            gel_p = ep(tc.tile_pool(name="gel", bufs=16))     # fp8
            wqk_p = ep(tc.tile_pool(name="wqk", bufs=4))      # [128,2048] fp8
            wvo_p = ep(tc.tile_pool(name="wvo", bufs=4))      # [128,2064] fp8
            w1_p = ep(tc.tile_pool(name="w1p", bufs=4))       # [128,2048] fp8
            w2_p = ep(tc.tile_pool(name="w2p", bufs=17))      # [128,512] fp8
            row_p = ep(tc.tile_pool(name="row", bufs=1))      # [1,2576] fp16
            bc_p = ep(tc.tile_pool(name="bc", bufs=2))        # [128,32] fp32
            lnr_p = ep(tc.tile_pool(name="lnr", bufs=2))      # [1,1024] fp32
            gb_p = ep(tc.tile_pool(name="gb", bufs=2))        # LN G/B broadcast fp32
            sc_p = ep(tc.tile_pool(name="sc", bufs=3))        # fp32 scratch
            ats_p = ep(tc.tile_pool(name="ats", bufs=3))      # [64,512] fp32
            s1_p = ep(tc.tile_pool(name="s1", bufs=2))        # [1,512] rows
            st_p = ep(tc.tile_pool(name="st", bufs=6))        # small stats
            ps_p = ep(tc.tile_pool(name="ps", bufs=4, space="PSUM"))
            ps2_p = ep(tc.tile_pool(name="ps2", bufs=2, space="PSUM"))
            dram_p = ep(tc.tile_pool(name="dram", bufs=4, space="DRAM"))

            identt = const_p.tile([128, 128], FP16, tag="ident", name="ident")
            nc.sync.dma_start(identt[:], ident_d[:])
            onest = const_p.tile([1, 128], FP16, tag="ones", name="ones")
            nc.sync.dma_start(onest[:], ones_d[:])
            knowfm = []
            for e in range(ET):
                t = know_p.tile([128, SK], FP8, tag="know", name="know")
                nc.sync.dma_start(t[:], know_fm_d[e * 128:(e + 1) * 128, :])
                knowfm.append(t)

            hid = []
            for t in range(TT):
                h = stm_p.tile([128, E], FP16, tag="stm", name="stm")
                nc.sync.dma_start(h[:], own_tm0[t * 128:(t + 1) * 128, :])
                hid.append(h)
            ownfm = []
            for e in range(ET):
                t = ofm_p.tile([128, CH], FP16, tag="ofm", name="ofm")
                nc.sync.dma_start(t[:], own_fm0[e * 128:(e + 1) * 128, :])
                ownfm.append(t)

            wts = {}

            def get_wts(l):
                if l in wts:
                    return wts[l]
                w = {}
                w["qk"] = []
                for ei in range(ET):
                    t = wqk_p.tile([128, 4 * E], FP16, tag="wqk", name="wqk")
                    nc.sync.dma_start(t[:], wqk_d[l, ei])
                    w["qk"].append(t)
                w["vo"] = []
                for ei in range(ET):
                    t = wvo_p.tile([128, 2 * HW * H + 2 * E], FP16, tag="wvo",
                                   name="wvo")
                    nc.sync.dma_start(t[:], wvo_d[l, ei])
                    w["vo"].append(t)
                w["w1"] = []
                for ei in range(ET):
                    t = w1_p.tile([128, F], FP16, tag="w1", name="w1")
                    nc.sync.dma_start(t[:], w1_d[l, ei])
                    w["w1"].append(t)
                w["w2"] = []
                for ft in range(FT):
                    t = w2_p.tile([128, E], FP16, tag="w2", name="w2")
                    nc.sync.dma_start(t[:], w2_d[l, ft])
                    w["w2"].append(t)
                w["rows"] = row_p.tile([1, 2 * HW * H + 3 * E], FP16, tag="row",
                                       name="row")
                nc.sync.dma_start(w["rows"][:], rows_d[l])
                w["bc"] = bc_p.tile([128, 4 * ET + FT], FP32, tag="bc", name="bc")
                nc.sync.dma_start(w["bc"][:], bc_d[l])
                w["lnr"] = lnr_p.tile([1, 2 * E], FP16, tag="lnr", name="lnr")
                nc.sync.dma_start(w["lnr"][:], lnr_d[l])
                w["G"] = gb_p.tile([128, E], FP16, tag="G", name="G")
                nc.gpsimd.partition_broadcast(w["G"][:], w["lnr"][:, 0:E])
                w["B"] = gb_p.tile([128, E], FP16, tag="B", name="B")
                nc.gpsimd.partition_broadcast(w["B"][:], w["lnr"][:, E:2 * E])
                wts[l] = w
                return w

            def ln_norm(xres, G, Bt, out):
                """out = G*(xres-mean)/(sqrt(bessel_var)+eps) + Bt, rows of 512."""
                stt = st_p.tile([128, 6], FP32, tag="bnst", name="bnst")
                nc.vector.bn_stats(out=stt[:], in_=xres[:])
                mv = st_p.tile([128, 2], FP32, tag="bnmv", name="bnmv")
                nc.vector.bn_aggr(out=mv[:], in_=stt[:])
                sd = st_p.tile([128, 1], FP32, tag="sd", name="sd")
                nc.scalar.activation(sd[:], mv[:, 1:2], AF.Sqrt, scale=float(E) / (E - 1))
                nc.vector.tensor_scalar_add(sd[:], sd[:], 1e-6)
                inv = st_p.tile([128, 1], FP32, tag="inv", name="inv")
                nc.vector.reciprocal_approx_fast(inv[:], sd[:])
                minv = st_p.tile([128, 1], FP32, tag="minv", name="minv")
                nc.vector.tensor_mul(minv[:], mv[:, 0:1], inv[:])
                nc.vector.tensor_scalar(xres[:], in0=xres[:], scalar1=inv[:],
                                        scalar2=minv[:], op0=OP.mult, op1=OP.subtract)
                nc.vector.tensor_mul(xres[:], xres[:], G[:])
                nc.vector.tensor_add(out[:], xres[:], Bt[:])

            def transpose_to(dst_tiles, src_tile, t):
                """src [128tok, E] TM tile t -> dst_tiles[e][:, t*128:(t+1)*128]."""
                for e in range(ET):
                    tp = ps_p.tile([128, 128], FP16, tag="ps", name="ps")
                    nc.tensor.transpose(tp[:], src_tile[:, e * 128:(e + 1) * 128],
                                        identt[:])
                    nc.vector.tensor_copy(dst_tiles[e][:, t * 128:(t + 1) * 128], tp[:])

            def q_proj(w, qoff, boff, src_fm):
                """Q (fp16 FM) from fp8 weights + fp16 FM activations."""
                q = []
                for e in range(ET):
                    pst = ps_p.tile([128, 512], FP32, tag="ps", name="ps")
                    for ei in range(ET):
                        nc.tensor.matmul(
                            pst[:], w["qk"][ei][:, qoff + e * 128:qoff + (e + 1) * 128],
                            src_fm[ei][:], start=(ei == 0), stop=(ei == ET - 1))
                    qt = qfm_p.tile([128, 512], FP16, tag="qfm", name="qfm")
                    nc.vector.tensor_scalar(qt[:], in0=pst[:], scalar1=IWS,
                                            scalar2=w["bc"][:, boff + e:boff + e + 1],
                                            op0=OP.mult, op1=OP.add)
                    q.append(qt)
                return q

            def k_proj(w, koff, boff, src_fm, kdst, col0, nch, pool=None):
                """K fp8 columns [col0, col0+nch*512) of kdst from FM src."""
                for e in range(ET):
                    for c2 in range(nch):
                        pst = (pool or ps_p).tile([128, 512], FP32, tag="psk"
                                                  if pool else "ps", name="ps")
                        for ei in range(ET):
                            nc.tensor.matmul(
                                pst[:],
                                w["qk"][ei][:, koff + e * 128:koff + (e + 1) * 128],
                                src_fm[ei][:, c2 * 512:(c2 + 1) * 512],
                                start=(ei == 0), stop=(ei == ET - 1))
                        nc.vector.tensor_scalar(
                            kdst[e][:, col0 + c2 * 512:col0 + (c2 + 1) * 512],
                            in0=pst[:], scalar1=IWS,
                            scalar2=w["bc"][:, boff + e:boff + e + 1],
                            op0=OP.mult, op1=OP.add)

            def v_proj(w, voff, rvoff, src_fm, vdst, kt0, nkt):
                """V' fp8, written into kt-pair slots of [128,2,528] tiles."""
                for ktl in range(nkt):
                    g = kt0 + ktl
                    vt, ko = vdst[g // 2], g % 2
                    for half in range(2):
                        cs = half * (H * HW // 2)
                        pst = ps_p.tile([128, H * HW // 2], FP32, tag="ps", name="ps")
                        for ei in range(ET):
                            nc.tensor.matmul(
                                pst[:], src_fm[ei][:, ktl * 128:(ktl + 1) * 128],
                                w["vo"][ei][:, voff + cs:voff + cs + H * HW // 2],
                                start=(ei == 0), stop=False)
                        nc.tensor.matmul(
                            pst[:], onest[:],
                            w["rows"][:, rvoff + cs:rvoff + cs + H * HW // 2],
                            start=False, stop=True)
                        nc.vector.tensor_scalar_mul(vt[:, ko, cs:cs + H * HW // 2],
                                                    pst[:], IWS)

            def attention(qfm, kfm, vpp, nkt, attn_tiles):
                np_ = nkt // 2
                for hs in range(2):
                    attps = [ps_p.tile([HW, 512], FP32, tag="ps", name="ps")
                             for _ in range(4)]
                    ptp = None
                    for kt in range(nkt):
                        if kt % 2 == 0:
                            ptp = [pt_p.tile([128, 2, 1024], FP8, tag="pt",
                                             name="pt") for _ in range(2)]
                        for half in range(2):
                            ps2 = ps2_p.tile([128, 1024], FP32, tag="ps2", name="ps2")
                            for j in range(2):
                                h = hs * 4 + half * 2 + j
                                e, r = h // 2, (h % 2) * 64
                                nc.tensor.matmul(
                                    ps2[:, j * 512:(j + 1) * 512],
                                    kfm[e][r:r + 64, kt * 128:(kt + 1) * 128],
                                    qfm[e][r:r + 64, :], start=True, stop=True)
                            nc.scalar.activation(ptp[half][:, kt % 2], ps2[:],
                                                 AF.Exp, scale=0.125)
                        if kt % 2 == 1:
                            p = kt // 2
                            for h4 in range(4):
                                h = hs * 4 + h4
                                nc.tensor.matmul(
                                    attps[h4][:],
                                    vpp[p][:, :, h * HW:(h + 1) * HW],
                                    ptp[h4 // 2][:, :, (h4 % 2) * 512:(h4 % 2 + 1) * 512],
                                    start=(p == 0), stop=(p == np_ - 1),
                                    perf_mode=mybir.MatmulPerfMode.DoubleRow)
                    for h4 in range(4):
                        h = hs * 4 + h4
                        e, r = h // 2, (h % 2) * 64
                        ats = ats_p.tile([64, 512], FP32, tag="ats", name="ats")
                        nc.vector.tensor_copy(ats[:], attps[h4][0:64, :])
                        den = s1_p.tile([1, 512], FP32, tag="den", name="den")
                        nc.vector.tensor_copy(den[:], attps[h4][64:65, :])
                        rec = s1_p.tile([1, 512], FP32, tag="rec", name="rec")
                        nc.vector.reciprocal_approx_fast(rec[:], den[:])
                        rb = sc_p.tile([64, 512], FP32, tag="rb", name="rb")
                        nc.gpsimd.partition_broadcast(rb[:], rec[:])
                        nc.vector.tensor_mul(attn_tiles[e][r:r + 64, :],
                                             ats[:], rb[:])

            def out_proj_ln(attn_tiles, w, ooff, rooff, res_tiles, G, Bt, out_tiles):
                for t in range(TT):
                    pst = ps_p.tile([128, E], FP32, tag="ps", name="ps")
                    for ei in range(ET):
                        nc.tensor.matmul(pst[:], attn_tiles[ei][:, t * 128:(t + 1) * 128],
                                         w["vo"][ei][:, ooff:ooff + E],
                                         start=(ei == 0), stop=False)
                    nc.tensor.matmul(pst[:], onest[:], w["rows"][:, rooff:rooff + E],
                                     start=False, stop=True)
                    xres = sc_p.tile([128, E], FP32, tag="xres", name="xres")
                    nc.vector.scalar_tensor_tensor(
                        xres[:], in0=pst[:], scalar=IWS, in1=res_tiles[t][:],
                        op0=OP.mult, op1=OP.add)
                    ln_norm(xres, G, Bt, out_tiles[t])

            def make_ca_k(l):
                w = get_wts(l)
                kca = [kca_p.tile([128, SK], FP8, tag="kca", name="kca")
                       for _ in range(ET)]
                k_proj(w, KC, BKC, knowfm, kca, 0, 2)
                return kca

            def make_ca_v(l):
                w = get_wts(l)
                vp_ca = [vp_p.tile([128, 2, 528], FP8, tag="vp", name="vp")
                         for _ in range(KT_CA // 2)]
                v_proj(w, VC, RVC, knowfm, vp_ca, 0, KT_CA)
                return vp_ca

            def kv_own_and_ag(l, xfm):
                """Own-chunk SA K/V (fp8) -> DRAM -> AllGather. Returns
                (ksa tiles, vp list, ag_out) with only own block filled."""
                w = get_wts(l)
                ksa = [kfm_p.tile([128, S], FP8, tag="kfm", name="kfm")
                       for _ in range(ET)]
                vpsa = [vp_p.tile([128, 2, 528], FP8, tag="vp", name="vp")
                        for _ in range(KT_SA // 2)]
                ag_in = dram_p.tile([CH, AGW], FP8, tag="agin", name="agin")
                # own K columns + stage to DRAM
                k_proj(w, KS, BKS, xfm, ksa, 0, 1)  # writes cols [0,512) temp
                # NOTE: own chunk id differs per core; K own block must land at
                # cols [ch*512, ...). We instead compute into cols [0,512) and
                # fix placement below via per-core ag layout: the AG output is
                # ordered by rank, so loads put every block (incl. own) at the
                # right columns. The temp cols [0,512) are overwritten by the
                # rank-0 block load unless this core IS rank 0 - so stage own K
                # to DRAM first, then reload all four blocks from ag_out.
                for e in range(ET):
                    nc.sync.dma_start(ag_in[e * 128:(e + 1) * 128, 0:E],
                                      ksa[e][:, 0:E])
                v_proj(w, VS, RVS, xfm, vpsa, 0, TT)  # own V tiles 0..3 temp
                for ktl in range(TT):
                    nc.sync.dma_start(ag_in[ktl * 128:(ktl + 1) * 128, E:AGW],
                                      vpsa[ktl // 2][:, ktl % 2, 0:H * HW])
                ag_out = dram_p.tile([S, AGW], FP8, tag="agout", name="agout")
                nc.gpsimd.collective_compute(
                    "AllGather", OP.bypass, replica_groups=GROUPS,
                    ins=[ag_in.opt()], outs=[ag_out.opt()])
                return ksa, vpsa, ag_out

            def kv_loads(ksa, vpsa, ag_out):
                """Fill all 4 chunks of K columns / V tiles from ag_out."""
                for c in range(4):
                    for e in range(ET):
                        nc.sync.dma_start(
                            ksa[e][:, c * 512:(c + 1) * 512],
                            ag_out[c * 512 + e * 128:c * 512 + (e + 1) * 128, 0:E])
                    for ktl in range(TT):
                        g = c * TT + ktl
                        nc.sync.dma_start(
                            vpsa[g // 2][:, g % 2, 0:H * HW],
                            ag_out[c * 512 + ktl * 128:c * 512 + (ktl + 1) * 128,
                                   E:AGW])

            # ---- layer 0 boundary work ----
            w0 = get_wts(0)
            ksa, vpsa, ag_out = kv_own_and_ag(0, ownfm)
            ca_kv = (make_ca_k(0), make_ca_v(0))
            qsa = q_proj(w0, QS, BQS, ownfm)
            kv_loads(ksa, vpsa, ag_out)

            for l in range(L):
                with nc.named_scope(f"L{l}"):
                    w = get_wts(l)
                    kca, vp_ca = ca_kv
                    G, Bt = w["G"], w["B"]

                    # ---- SA attention + out-proj + LN1 ----
                    attn = [attn_p.tile([128, 512], FP16, tag="attn", name="attn")
                            for _ in range(ET)]
                    attention(qsa, ksa, vpsa, KT_SA, attn)
                    inter = [stm_p.tile([128, E], FP16, tag="stm", name="stm")
                             for _ in range(TT)]
                    out_proj_ln(attn, w, OS, ROS, hid, G, Bt, inter)

                    interfm = [xfm_p.tile([128, CH], FP16, tag="xfm", name="xfm")
                               for _ in range(ET)]
                    for t in range(TT):
                        transpose_to(interfm, inter[t], t)

                    # ---- CA Q + attention + out-proj + LN2 ----
                    qca = q_proj(w, QC, BQC, interfm)
                    attn2 = [attn_p.tile([128, 512], FP16, tag="attn", name="attn")
                             for _ in range(ET)]
                    attention(qca, kca, vp_ca, KT_CA, attn2)
                    co = [stm_p.tile([128, E], FP16, tag="stm", name="stm")
                          for _ in range(TT)]
                    out_proj_ln(attn2, w, OC, ROC, inter, G, Bt, co)

                    cofm = [xfm_p.tile([128, CH], FP16, tag="xfm", name="xfm")
                            for _ in range(ET)]
                    for t in range(TT):
                        transpose_to(cofm, co[t], t)

                    # ---- FFN ----
                    gel = []
                    for ft in range(FT):
                        pst = ps_p.tile([128, 512], FP32, tag="ps", name="ps")
                        for ei in range(ET):
                            nc.tensor.matmul(
                                pst[:], w["w1"][ei][:, ft * 128:(ft + 1) * 128],
                                cofm[ei][:], start=(ei == 0), stop=(ei == ET - 1))
                        gt = gel_p.tile([128, 512], FP16, tag="gel", name="gel")
                        nc.scalar.activation(gt[:], pst[:], AF.Gelu,
                                             bias=w["bc"][:, B1 + ft:B1 + ft + 1],
                                             scale=IWS)
                        gel.append(gt)
                    h2ps = [ps_p.tile([128, E], FP32, tag="ps", name="ps")
                            for _ in range(TT)]
                    for t in range(TT):
                        for ft in range(FT):
                            nc.tensor.matmul(h2ps[t][:], gel[ft][:, t * 128:(t + 1) * 128],
                                             w["w2"][ft][:], start=(ft == 0), stop=False)
                    hidn = [stm_p.tile([128, E], FP16, tag="stm", name="stm")
                            for _ in range(TT)]
                    for t in range(TT):
                        nc.tensor.matmul(h2ps[t][:], onest[:],
                                         w["rows"][:, R2:R2 + E],
                                         start=False, stop=True)
                        xres = sc_p.tile([128, E], FP32, tag="xres", name="xres")
                        nc.vector.scalar_tensor_tensor(
                            xres[:], in0=h2ps[t][:], scalar=IWS, in1=co[t][:],
                            op0=OP.mult, op1=OP.add)
                        ln_norm(xres, G, Bt, hidn[t])
                        if l == L - 1:
                            nc.sync.dma_start(out_d[t * 128:(t + 1) * 128, :], hidn[t][:])

                    if l < L - 1:
                        ownfm_n = [ofm_p.tile([128, CH], FP16, tag="ofm", name="ofm")
                                   for _ in range(ET)]
                        for t in range(TT):
                            transpose_to(ownfm_n, hidn[t], t)
                        wn = get_wts(l + 1)
                        ksa, vpsa, ag_out = kv_own_and_ag(l + 1, ownfm_n)
                        # AG-latency fillers:
                        ca_kv = (make_ca_k(l + 1), make_ca_v(l + 1))
                        qsa = q_proj(wn, QS, BQS, ownfm_n)
                        kv_loads(ksa, vpsa, ag_out)
                        ownfm = ownfm_n
                        hid = hidn

    nc.compile()
    return nc


def _prep_inputs(sen, know, sa_qkv_w, sa_qkv_b, sa_out_w, sa_out_b,
                 ca_qkv_w, ca_qkv_b, ca_out_w, ca_out_b,
                 ff_w1, ff_b1, ff_w2, ff_b2, ln_g, ln_b):
    """Host-side weight packing shared by all cores + per-core activations."""
    f16, f32 = np.float16, np.float32
    f8 = mybir.dt.np(FP8)

    def blk(w):  # [L,E,X] -> [L,ET,128,X] fp16
        return (w.reshape(L, ET, 128, -1)).astype(f16)

    def padv(w):  # [L,E,E] -> [L,ET,128,H*HW] (no bias; scaled)
        wp = np.zeros((L, E, H, HW), f32)
        wp[:, :, :, :D] = w.reshape(L, E, H, D)
        return (wp.reshape(L, ET, 128, H * HW)).astype(f16)

    def rowv(b):  # [L,E] -> [L,1,H*HW] fp16 scaled, with ones col
        bp = np.zeros((L, H, HW), f32)
        bp[:, :, :D] = b.reshape(L, H, D)
        bp[:, :, D] = 1.0
        return (bp.reshape(L, 1, H * HW) * WS).astype(f16)

    wqk = np.concatenate([blk(sa_qkv_w[:, 0]), blk(sa_qkv_w[:, 1]),
                          blk(ca_qkv_w[:, 0]), blk(ca_qkv_w[:, 1])], axis=3)
    wvo = np.concatenate([padv(sa_qkv_w[:, 2]), padv(ca_qkv_w[:, 2]),
                          blk(sa_out_w), blk(ca_out_w)], axis=3)
    rows = np.concatenate([rowv(sa_qkv_b[:, 2]), rowv(ca_qkv_b[:, 2]),
                           (sa_out_b[:, None, :] * WS).astype(f16),
                           (ca_out_b[:, None, :] * WS).astype(f16),
                           (ff_b2[:, None, :] * WS).astype(f16)], axis=2)

    def bcol(b):  # [L,X*128] -> [L,128,X]
        return np.ascontiguousarray(
            b.reshape(L, -1, 128).transpose(0, 2, 1)).astype(f32)

    bc = np.concatenate([bcol(sa_qkv_b[:, 0]), bcol(sa_qkv_b[:, 1]),
                         bcol(ca_qkv_b[:, 0]), bcol(ca_qkv_b[:, 1]),
                         bcol(ff_b1)], axis=2)
    lnr = np.concatenate([ln_g[:, None, :], ln_b[:, None, :]], axis=2).astype(f16)

    common = {
        "ident": np.eye(128, dtype=f16),
        "ones": np.ones((1, 128), f16),
        "wqk": np.ascontiguousarray(wqk),
        "wvo": np.ascontiguousarray(wvo),
        "w1": np.ascontiguousarray(blk(ff_w1)),
        "w2": np.ascontiguousarray((ff_w2.reshape(L, FT, 128, E)).astype(f16)),
        "rows": np.ascontiguousarray(rows),
        "bc": np.ascontiguousarray(bc),
        "lnr": np.ascontiguousarray(lnr),
    }
    in_maps = []
    for core in range(NCORES):
        g, c = core // 4, core % 4
        m = dict(common)
        m["own_fm0"] = np.ascontiguousarray(sen[g, c * CH:(c + 1) * CH].T.astype(f16))
        m["own_tm0"] = np.ascontiguousarray(sen[g, c * CH:(c + 1) * CH].astype(f16))
        m["know_fm"] = np.ascontiguousarray(know[g].T.astype(f8))
        in_maps.append(m)
    return in_maps


def kernel(**inputs):
    inputs = {k: np.asarray(v, dtype=np.float32) for k, v in inputs.items()}
    if "nc" not in _CACHE:
        _CACHE["nc"] = _build()
    nc = _CACHE["nc"]
    in_maps = _prep_inputs(**inputs)
    res = run_bass_kernel_spmd(nc, in_maps, list(range(NCORES)))
    out = np.empty((B, S, E), np.float32)
    for core in range(NCORES):
        g, c = core // 4, core % 4
        out[g, c * CH:(c + 1) * CH] = res.results[core]["out_tm"]
    return out
